# revision 5
# baseline (speedup 1.0000x reference)
"""Trainium2 Bass kernel for the directional min-variance filter (Kuwahara-style).

Row-per-partition, PE-centric design. The 1024x1024 image is processed in 9
stripes of 114 output rows. Per stripe, a 128-row x tile is DMA-loaded three
ways from a host-padded DRAM image: straight, sheared +1 col/row, sheared -1
col/row. A banded 128x121 fp32 matmul (8-ones diagonal band) computes 8-pixel
window sums along the partition axis, which in the three layouts realizes the
vertical, diagonal, and anti-diagonal ray sums of x and x^2 on the otherwise
idle Tensor engine. Horizontal ray sums run on the Vector engine via log2
doubling on a separate 114-row tile. Sheared results are un-sheared by a DMA
roundtrip through DRAM (parallelogram write, straight read), which also
provides the two partition origins (ray rows r0-7.. and r0..) that the
selection step needs, since compute engines require partition-0-aligned
accesses.

Metric per forward direction i: m_i = x1_i^2/8 - y2_i (maximize == minimize
variance); backward rays reuse the forward buffers via shifted views.
Selection: best metric M via a 7-op max tree, then first-wins payload select
with is_equal masks + copy_predicated in reverse direction order (exact
argmin tie semantics). 8 NeuronCores run data-parallel over the 8 images.
"""

import numpy as np

import concourse.bass as bass
import concourse.bacc as bacc
import concourse.tile as tile
from concourse import mybir
from concourse.bass_utils import run_bass_kernel_spmd

F32 = mybir.dt.float32
U8 = mybir.dt.uint8

W = 1024
SOUT = 114                  # output rows per stripe (last stripe: 112)
NS = 9
NQ = 121                    # metric rows per stripe (ray rows r0-7 .. r0+113)
NO = 114                    # chain rows per stripe
PADL = 144
PADR = 152
WPAD = PADL + W + PADR      # 1320
HPAD = 1040                 # xpad rows: image rows -7 .. 1032
WS = W + 16                 # straight tiles: col j <-> image col j-8
WZ = W + 136                # sheared tiles
WSCR = 1312                 # DRAM unshear scratch row pitch (>= NQ + WZ + 24)

MAX = mybir.AluOpType.max
EQ = mybir.AluOpType.is_equal
MUL = mybir.AluOpType.mult
SUB = mybir.AluOpType.subtract


def _build():
    nc = bacc.Bacc("TRN2", target_bir_lowering=False)
    x_t = nc.declare_dram_parameter("x", [HPAD, WPAD], F32, isOutput=False)
    wb_t = nc.declare_dram_parameter("wb", [128, NQ], F32, isOutput=False)
    y_t = nc.declare_dram_parameter("y", [W, W], F32, isOutput=True)
    scr_t = nc.dram_tensor("scr", [4, NQ, WSCR], F32, kind="Internal")
    xp = x_t[:]
    yp = y_t[:]
    scr = scr_t[:]

    def sbase(bi):
        return scr.offset + bi * NQ * WSCR

    with tile.TileContext(nc) as tc:
        with tc.tile_pool(name="const", bufs=1) as pc, \
             tc.tile_pool(name="pin", bufs=2) as pin, \
             tc.tile_pool(name="pres", bufs=1) as pres, \
             tc.tile_pool(name="pscr", bufs=1) as pscr, \
             tc.tile_pool(name="pout", bufs=2) as pout, \
             tc.tile_pool(name="pps", bufs=1, space="PSUM") as pps:

            band = pc.tile([128, NQ], F32)
            nc.sync.dma_start(out=band[:], in_=wb_t[:])

            SC1 = pscr.tile([NO, WS], F32)
            SC2 = pscr.tile([NO, WS], F32)
            Y2A = pscr.tile([NO, WS], F32)
            T0 = pscr.tile([NO, W], F32)
            T1 = pscr.tile([NO, W], F32)
            MBD = pscr.tile([NO, W], F32)
            MK0 = pscr.tile([NO, W], U8)
            MK1 = pscr.tile([NO, W], U8)

            def loads(s):
                r0 = s * SOUT
                t = {}
                t["xs"] = pin.tile([128, WS], F32, name=f"xs{s}", tag="xs")
                t["xs2"] = pin.tile([NO, WS], F32, name=f"xs2{s}", tag="xs2")
                t["zp"] = pin.tile([128, WZ], F32, name=f"zp{s}", tag="zp")
                t["zm"] = pin.tile([128, WZ], F32, name=f"zm{s}", tag="zm")
                for h0, h1 in ((0, 64), (64, 128)):
                    nc.sync.dma_start(
                        out=t["xs"][h0:h1, :], in_=bass.AP(
                            tensor=xp.tensor,
                            offset=xp.offset + (r0 + h0) * WPAD + PADL - 8,
                            ap=[[WPAD, h1 - h0], [1, WS]]))
                nc.sync.dma_start(
                    out=t["xs2"][:], in_=bass.AP(
                        tensor=xp.tensor,
                        offset=xp.offset + (r0 + 7) * WPAD + PADL - 8,
                        ap=[[WPAD, NO], [1, WS]]))
                for h0, h1 in ((0, 64), (64, 128)):
                    nc.sync.dma_start(
                        out=t["zp"][h0:h1, :], in_=bass.AP(
                            tensor=xp.tensor,
                            offset=(xp.offset + r0 * WPAD + h0 * (WPAD + 1)
                                    + PADL - 128),
                            ap=[[WPAD + 1, h1 - h0], [1, WZ]]))
                    nc.sync.dma_start(
                        out=t["zm"][h0:h1, :], in_=bass.AP(
                            tensor=xp.tensor,
                            offset=(xp.offset + r0 * WPAD + h0 * (WPAD - 1)
                                    + PADL - 8),
                            ap=[[WPAD - 1, h1 - h0], [1, WZ]]))
                return t

            def stripe(s, t, dbl_done):
                xs, xs2, zp, zm = t["xs"], t["xs2"], t["zp"], t["zm"]

                def rt(nm, p, w):
                    return pres.tile([p, w], F32, name=f"{nm}{s}", tag=nm)

                x1a, ma = dbl_done  # from emit_doubling
                x1b_u = rt("x1b_u", NQ, W)
                mb_u = rt("mb_u", NQ, W)
                x1b_d = rt("x1b_d", NO, W)
                x1c_u = rt("x1c_u", NO, W)
                mc_u = rt("mc_u", NO, W)
                x1c_d = rt("x1c_d", NO, W)
                mc_d = rt("mc_d", NO, W)
                x1e_u = rt("x1e_u", NO, W)
                me_u = rt("me_u", NO, W)
                x1e_d = rt("x1e_d", NO, W)
                me_d = rt("me_d", NO, W)
                x1sh = pres.tile([NQ, WZ], F32, name=f"x1sh{s}", tag="x1sh", bufs=2)
                msh = pres.tile([NQ, WZ], F32, name=f"msh{s}", tag="msh", bufs=2)
                x1sh2 = pres.tile([NQ, WZ], F32, name=f"x1sh2{s}", tag="x1sh2", bufs=2)
                msh2 = pres.tile([NQ, WZ], F32, name=f"msh2{s}", tag="msh2", bufs=2)

                # --- sheared dirs first (c: +1, e: -1); chunked roundtrip ---
                for k, zt, x1t, mt, sb, sadj, woff, rds in (
                        ("c", zp, x1sh, msh, 0, 1, 0,
                         ((1, 0, 121), (1, 7, 128), (0, 0, 121), (0, 7, 128))),
                        ("e", zm, x1sh2, msh2, 2, -1, 120,
                         ((3, 0, 135), (3, 7, 128), (2, 0, 135), (2, 7, 128)))):
                    for c0 in (0, 512, 1024):
                        cw = min(512, WZ - c0)
                        p1 = pps.tile([NQ, 512], F32, name=f"px{k}{s}{c0}",
                                      tag="psC", bufs=2)
                        nc.tensor.matmul(p1[:, :cw], band[:], zt[:, c0:c0 + cw],
                                         start=True, stop=True)
                        nc.scalar.copy(x1t[:, c0:c0 + cw], p1[:, :cw])
                        nc.scalar.square(mt[:, c0:c0 + cw], p1[:, :cw])
                        # x1 sheared chunk can roundtrip-write immediately
                        nc.gpsimd.dma_start(
                            out=bass.AP(tensor=scr.tensor,
                                        offset=sbase(sb) + woff + c0,
                                        ap=[[WSCR + sadj, NQ], [1, cw]]),
                            in_=x1t[:, c0:c0 + cw])
                    nc.scalar.square(zt[:], zt[:])  # in-place x^2
                    for c0 in (0, 512, 1024):
                        cw = min(512, WZ - c0)
                        p2 = pps.tile([NQ, 512], F32, name=f"py{k}{s}{c0}",
                                      tag="psD", bufs=2)
                        nc.tensor.matmul(p2[:, :cw], band[:], zt[:, c0:c0 + cw],
                                         start=True, stop=True)
                        nc.vector.scalar_tensor_tensor(
                            out=mt[:, c0:c0 + cw], in0=mt[:, c0:c0 + cw],
                            scalar=0.125, in1=p2[:, :cw], op0=MUL, op1=SUB)
                        nc.gpsimd.dma_start(
                            out=bass.AP(tensor=scr.tensor,
                                        offset=sbase(sb + 1) + woff + c0,
                                        ap=[[WSCR + sadj, NQ], [1, cw]]),
                            in_=mt[:, c0:c0 + cw])
                    # row-split straight reads (2-way DMA parallelism)
                    dsts = {("c"): (mc_u, mc_d, x1c_u, x1c_d),
                            ("e"): (me_u, me_d, x1e_u, x1e_d)}[k]
                    for dst, (bi_l, roff, coff) in zip(dsts, rds):
                        for rr0, rr1 in ((0, 57), (57, NO)):
                            nc.gpsimd.dma_start(
                                out=dst[rr0:rr1, :], in_=bass.AP(
                                    tensor=scr.tensor,
                                    offset=(sbase(bi_l) + (roff + rr0) * WSCR
                                            + coff),
                                    ap=[[WSCR, rr1 - rr0], [1, W]]))

                # --- vertical (dir b) on PE; xs squared in-place after x1 ---
                pxv = []
                for c0 in (0, 512):
                    p1 = pps.tile([NQ, 512], F32, name=f"pxv{s}{c0}",
                                  tag="psA", bufs=2)
                    nc.tensor.matmul(p1[:], band[:], xs[:, 8 + c0:8 + c0 + 512],
                                     start=True, stop=True)
                    nc.scalar.copy(x1b_u[:, c0:c0 + 512], p1[:])
                    nc.scalar.square(mb_u[:, c0:c0 + 512], p1[:])
                nc.scalar.square(xs[:], xs[:])
                for c0 in (0, 512):
                    p2 = pps.tile([NQ, 512], F32, name=f"pyv{s}{c0}",
                                  tag="psB", bufs=2)
                    nc.tensor.matmul(p2[:], band[:], xs[:, 8 + c0:8 + c0 + 512],
                                     start=True, stop=True)
                    nc.vector.scalar_tensor_tensor(
                        out=mb_u[:, c0:c0 + 512], in0=mb_u[:, c0:c0 + 512],
                        scalar=0.125, in1=p2[:], op0=MUL, op1=SUB)
                    nc.gpsimd.dma_start(out=x1b_d[:, c0:c0 + 512],
                                        in_=x1b_u[7:NQ, c0:c0 + 512])
                    nc.gpsimd.dma_start(out=MBD[:, c0:c0 + 512],
                                        in_=mb_u[7:NQ, c0:c0 + 512])

                # m_a from the early-emitted doubling results
                nc.scalar.square(ma[:, 1:1033], x1a[:, 1:1033])
                nc.vector.scalar_tensor_tensor(
                    out=ma[:, 1:1033], in0=ma[:, 1:1033], scalar=0.125,
                    in1=Y2A[:, 1:1033], op0=MUL, op1=SUB)

                return {
                    "views": [
                        (mc_u[:], x1c_u[:]),
                        (mb_u[0:NO, :], x1b_u[0:NO, :]),
                        (me_u[:], x1e_u[:]),
                        (ma[:, 1:1025], x1a[:, 1:1025]),
                        (ma[:, 8:1032], x1a[:, 8:1032]),
                        (me_d[:], x1e_d[:]),
                        (MBD[:], x1b_d[:]),
                        (mc_d[:], x1c_d[:]),
                    ]}

            def emit_doubling(s, t):
                xs2 = t["xs2"]
                x1a = pres.tile([NO, WS], F32, name=f"x1a{s}", tag="x1a",
                                bufs=2)
                ma = pres.tile([NO, WS], F32, name=f"ma{s}", tag="ma", bufs=2)
                # x1_a: prefix scan + shifted subtract (S col i = cols <= i-8)
                nc.vector.tensor_tensor_scan(
                    SC1[:], xs2[:], xs2[:], 0.0,
                    op0=mybir.AluOpType.add, op1=mybir.AluOpType.bypass)
                nc.vector.tensor_tensor(x1a[:, 1:1033], SC1[:, 8:1040],
                                        SC1[:, 0:1032], SUB)
                # y2_a: doubling (exact adds, metric-precision critical)
                nc.scalar.square(xs2[:], xs2[:])
                nc.vector.tensor_add(SC1[:, 1:1039], xs2[:, 1:1039],
                                     xs2[:, 2:1040])
                nc.vector.tensor_add(SC2[:, 1:1037], SC1[:, 1:1037],
                                     SC1[:, 3:1039])
                nc.vector.tensor_add(Y2A[:, 1:1033], SC2[:, 1:1033],
                                     SC2[:, 5:1037])
                return x1a, ma

            def chain(s, r):
                views = r["views"]
                # max tree: e-dependent results (d2, d5) joined last
                nc.vector.tensor_tensor(T0[:], views[3][0], views[4][0], MAX)
                nc.vector.tensor_tensor(T1[:], views[1][0], views[6][0], MAX)
                nc.vector.tensor_tensor(T0[:], T0[:], T1[:], MAX)
                nc.vector.tensor_tensor(T1[:], views[0][0], views[7][0], MAX)
                nc.vector.tensor_tensor(T0[:], T0[:], T1[:], MAX)
                nc.vector.tensor_tensor(T1[:], views[2][0], views[5][0], MAX)
                nc.vector.tensor_tensor(T0[:], T0[:], T1[:], MAX)

                bx = pout.tile([NO, W], F32, name=f"bx{s}", tag="bx")
                outt = pout.tile([NO, W], F32, name=f"out{s}", tag="outt")
                nc.scalar.copy(bx[:], views[7][1])
                for d in range(6, -1, -1):
                    mk = (MK0 if d % 2 == 0 else MK1)[:]
                    nc.vector.tensor_tensor(mk, views[d][0], T0[:], EQ)
                    nc.vector.copy_predicated(bx[:], mk, views[d][1])
                nc.scalar.mul(outt[:], bx[:], 0.125)
                nout = SOUT if s < NS - 1 else W - (NS - 1) * SOUT
                nc.sync.dma_start(
                    out=bass.AP(tensor=yp.tensor,
                                offset=yp.offset + s * SOUT * W,
                                ap=[[W, nout], [1, W]]),
                    in_=outt[0:nout, :])

            t = loads(0)
            dbl = emit_doubling(0, t)
            res = None
            for s in range(NS):
                t_next = loads(s + 1) if s + 1 < NS else None
                res = stripe(s, t, dbl)
                if t_next is not None:
                    dbl = emit_doubling(s + 1, t_next)
                chain(s, res)
                t = t_next
    nc.compile()
    return nc


def _make_band():
    wb = np.zeros((128, NQ), np.float32)
    for q in range(NQ):
        wb[q:q + 8, q] = 1.0
    return wb


_nc_cache = []


def _get_nc():
    if not _nc_cache:
        _nc_cache.append(_build())
    return _nc_cache[0]


def _in_maps(x):
    wb = _make_band()
    maps = []
    for i in range(x.shape[0]):
        xpad = np.zeros((HPAD, WPAD), np.float32)
        xpad[7:7 + W, PADL:PADL + W] = x[i, 0]
        maps.append({"x": xpad, "wb": wb})
    return maps


def kernel(x, weight=None, _want_results=False, **_ignored):
    x = np.ascontiguousarray(np.asarray(x), dtype=np.float32)
    n = x.shape[0]
    assert x.shape == (n, 1, W, W), x.shape
    nc = _get_nc()
    res = run_bass_kernel_spmd(nc, _in_maps(x), core_ids=list(range(n)))
    out = np.stack([r["y"] for r in res.results])[:, None]
    if _want_results:
        return out, res
    return out


if __name__ == "__main__":
    rng = np.random.default_rng(0)
    x = rng.standard_normal((8, 1, W, W)).astype(np.float32)
    y = kernel(x)
    print("ran; out shape", y.shape, "mean", y.mean())


# revision 6
# speedup vs baseline: 1.0080x; 1.0080x over previous
"""Trainium2 Bass kernel for the directional min-variance filter (Kuwahara-style).

Row-per-partition, PE-centric design. The 1024x1024 image is processed in 9
stripes of 114 output rows. Per stripe, a 128-row x tile is DMA-loaded three
ways from a host-padded DRAM image: straight, sheared +1 col/row, sheared -1
col/row. A banded 128x121 fp32 matmul (8-ones diagonal band) computes 8-pixel
window sums along the partition axis, which in the three layouts realizes the
vertical, diagonal, and anti-diagonal ray sums of x and x^2 on the otherwise
idle Tensor engine. Horizontal ray sums run on the Vector engine via log2
doubling on a separate 114-row tile. Sheared results are un-sheared by a DMA
roundtrip through DRAM (parallelogram write, straight read), which also
provides the two partition origins (ray rows r0-7.. and r0..) that the
selection step needs, since compute engines require partition-0-aligned
accesses.

Metric per forward direction i: m_i = x1_i^2/8 - y2_i (maximize == minimize
variance); backward rays reuse the forward buffers via shifted views.
Selection: best metric M via a 7-op max tree, then first-wins payload select
with is_equal masks + copy_predicated in reverse direction order (exact
argmin tie semantics). 8 NeuronCores run data-parallel over the 8 images.
"""

import numpy as np

import concourse.bass as bass
import concourse.bacc as bacc
import concourse.tile as tile
from concourse import mybir
from concourse.bass_utils import run_bass_kernel_spmd

F32 = mybir.dt.float32
U8 = mybir.dt.uint8

W = 1024
SOUT = 114                  # output rows per stripe (last stripe: 112)
NS = 9
NQ = 121                    # metric rows per stripe (ray rows r0-7 .. r0+113)
NO = 114                    # chain rows per stripe
PADL = 144
PADR = 152
WPAD = PADL + W + PADR      # 1320
HPAD = 1040                 # xpad rows: image rows -7 .. 1032
WS = W + 16                 # straight tiles: col j <-> image col j-8
WZ = W + 136                # sheared tiles
WSCR = 1312                 # DRAM unshear scratch row pitch (>= NQ + WZ + 24)

MAX = mybir.AluOpType.max
EQ = mybir.AluOpType.is_equal
MUL = mybir.AluOpType.mult
SUB = mybir.AluOpType.subtract


def _build():
    nc = bacc.Bacc("TRN2", target_bir_lowering=False)
    x_t = nc.declare_dram_parameter("x", [HPAD, WPAD], F32, isOutput=False)
    wb_t = nc.declare_dram_parameter("wb", [128, NQ], F32, isOutput=False)
    y_t = nc.declare_dram_parameter("y", [W, W], F32, isOutput=True)
    scr_t = nc.dram_tensor("scr", [4, NQ, WSCR], F32, kind="Internal")
    xp = x_t[:]
    yp = y_t[:]
    scr = scr_t[:]

    def sbase(bi):
        return scr.offset + bi * NQ * WSCR

    with tile.TileContext(nc) as tc:
        with tc.tile_pool(name="const", bufs=1) as pc, \
             tc.tile_pool(name="pin", bufs=2) as pin, \
             tc.tile_pool(name="pres", bufs=1) as pres, \
             tc.tile_pool(name="pscr", bufs=1) as pscr, \
             tc.tile_pool(name="pout", bufs=2) as pout, \
             tc.tile_pool(name="pps", bufs=1, space="PSUM") as pps:

            band = pc.tile([128, NQ], F32)
            nc.sync.dma_start(out=band[:], in_=wb_t[:])

            SC1 = pscr.tile([NO, WS], F32)
            SC2 = pscr.tile([NO, WS], F32)
            Y2A = pscr.tile([NO, WS], F32)
            T0 = pscr.tile([NO, W], F32)
            T1 = pscr.tile([NO, W], F32)
            MBD = pscr.tile([NO, W], F32)
            MK0 = pscr.tile([NO, W], U8)
            MK1 = pscr.tile([NO, W], U8)

            def loads(s):
                r0 = s * SOUT
                t = {}
                t["xs"] = pin.tile([128, WS], F32, name=f"xs{s}", tag="xs")
                t["xs2"] = pin.tile([NO, WS], F32, name=f"xs2{s}", tag="xs2")
                t["zp"] = pin.tile([128, WZ], F32, name=f"zp{s}", tag="zp")
                t["zm"] = pin.tile([128, WZ], F32, name=f"zm{s}", tag="zm")
                for h0, h1 in ((0, 64), (64, 128)):
                    nc.sync.dma_start(
                        out=t["xs"][h0:h1, :], in_=bass.AP(
                            tensor=xp.tensor,
                            offset=xp.offset + (r0 + h0) * WPAD + PADL - 8,
                            ap=[[WPAD, h1 - h0], [1, WS]]))
                nc.sync.dma_start(
                    out=t["xs2"][:], in_=bass.AP(
                        tensor=xp.tensor,
                        offset=xp.offset + (r0 + 7) * WPAD + PADL - 8,
                        ap=[[WPAD, NO], [1, WS]]))
                for h0, h1 in ((0, 64), (64, 128)):
                    nc.sync.dma_start(
                        out=t["zp"][h0:h1, :], in_=bass.AP(
                            tensor=xp.tensor,
                            offset=(xp.offset + r0 * WPAD + h0 * (WPAD + 1)
                                    + PADL - 128),
                            ap=[[WPAD + 1, h1 - h0], [1, WZ]]))
                    nc.sync.dma_start(
                        out=t["zm"][h0:h1, :], in_=bass.AP(
                            tensor=xp.tensor,
                            offset=(xp.offset + r0 * WPAD + h0 * (WPAD - 1)
                                    + PADL - 8),
                            ap=[[WPAD - 1, h1 - h0], [1, WZ]]))
                return t

            def stripe(s, t, dbl_done):
                xs, xs2, zp, zm = t["xs"], t["xs2"], t["zp"], t["zm"]

                def rt(nm, p, w):
                    return pres.tile([p, w], F32, name=f"{nm}{s}", tag=nm)

                x1a, ma = dbl_done  # from emit_doubling
                x1b_u = rt("x1b_u", NQ, W)
                mb_u = rt("mb_u", NQ, W)
                x1b_d = rt("x1b_d", NO, W)
                x1c_u = rt("x1c_u", NO, W)
                mc_u = rt("mc_u", NO, W)
                x1c_d = rt("x1c_d", NO, W)
                mc_d = rt("mc_d", NO, W)
                x1e_u = rt("x1e_u", NO, W)
                me_u = rt("me_u", NO, W)
                x1e_d = rt("x1e_d", NO, W)
                me_d = rt("me_d", NO, W)
                x1sh = pres.tile([NQ, WZ], F32, name=f"x1sh{s}", tag="x1sh", bufs=2)
                msh = pres.tile([NQ, WZ], F32, name=f"msh{s}", tag="msh", bufs=2)
                x1sh2 = pres.tile([NQ, WZ], F32, name=f"x1sh2{s}", tag="x1sh2", bufs=2)
                msh2 = pres.tile([NQ, WZ], F32, name=f"msh2{s}", tag="msh2", bufs=2)

                # --- sheared dirs first (c: +1, e: -1); chunked roundtrip ---
                for k, zt, x1t, mt, sb, sadj, woff, rds in (
                        ("c", zp, x1sh, msh, 0, 1, 0,
                         ((1, 0, 121), (1, 7, 128), (0, 0, 121), (0, 7, 128))),
                        ("e", zm, x1sh2, msh2, 2, -1, 120,
                         ((3, 0, 135), (3, 7, 128), (2, 0, 135), (2, 7, 128)))):
                    for c0 in (0, 512, 1024):
                        cw = min(512, WZ - c0)
                        p1 = pps.tile([NQ, 512], F32, name=f"px{k}{s}{c0}",
                                      tag="psC", bufs=2)
                        nc.tensor.matmul(p1[:, :cw], band[:], zt[:, c0:c0 + cw],
                                         start=True, stop=True)
                        nc.scalar.copy(x1t[:, c0:c0 + cw], p1[:, :cw])
                        nc.scalar.square(mt[:, c0:c0 + cw], p1[:, :cw])
                        # x1 sheared chunk can roundtrip-write immediately
                        nc.gpsimd.dma_start(
                            out=bass.AP(tensor=scr.tensor,
                                        offset=sbase(sb) + woff + c0,
                                        ap=[[WSCR + sadj, NQ], [1, cw]]),
                            in_=x1t[:, c0:c0 + cw])
                    nc.scalar.square(zt[:], zt[:])  # in-place x^2
                    for c0 in (0, 512, 1024):
                        cw = min(512, WZ - c0)
                        p2 = pps.tile([NQ, 512], F32, name=f"py{k}{s}{c0}",
                                      tag="psD", bufs=2)
                        nc.tensor.matmul(p2[:, :cw], band[:], zt[:, c0:c0 + cw],
                                         start=True, stop=True)
                        nc.vector.scalar_tensor_tensor(
                            out=mt[:, c0:c0 + cw], in0=mt[:, c0:c0 + cw],
                            scalar=0.125, in1=p2[:, :cw], op0=MUL, op1=SUB)
                        nc.gpsimd.dma_start(
                            out=bass.AP(tensor=scr.tensor,
                                        offset=sbase(sb + 1) + woff + c0,
                                        ap=[[WSCR + sadj, NQ], [1, cw]]),
                            in_=mt[:, c0:c0 + cw])
                    # row-split straight reads (2-way DMA parallelism)
                    dsts = {("c"): (mc_u, mc_d, x1c_u, x1c_d),
                            ("e"): (me_u, me_d, x1e_u, x1e_d)}[k]
                    for dst, (bi_l, roff, coff) in zip(dsts, rds):
                        for rr0, rr1 in ((0, 57), (57, NO)):
                            nc.gpsimd.dma_start(
                                out=dst[rr0:rr1, :], in_=bass.AP(
                                    tensor=scr.tensor,
                                    offset=(sbase(bi_l) + (roff + rr0) * WSCR
                                            + coff),
                                    ap=[[WSCR, rr1 - rr0], [1, W]]))

                # --- vertical (dir b) on PE; xs squared in-place after x1 ---
                pxv = []
                for c0 in (0, 512):
                    p1 = pps.tile([NQ, 512], F32, name=f"pxv{s}{c0}",
                                  tag="psA", bufs=2)
                    nc.tensor.matmul(p1[:], band[:], xs[:, 8 + c0:8 + c0 + 512],
                                     start=True, stop=True)
                    nc.scalar.copy(x1b_u[:, c0:c0 + 512], p1[:])
                    nc.scalar.square(mb_u[:, c0:c0 + 512], p1[:])
                nc.scalar.square(xs[:], xs[:])
                for c0 in (0, 512):
                    p2 = pps.tile([NQ, 512], F32, name=f"pyv{s}{c0}",
                                  tag="psB", bufs=2)
                    nc.tensor.matmul(p2[:], band[:], xs[:, 8 + c0:8 + c0 + 512],
                                     start=True, stop=True)
                    nc.vector.scalar_tensor_tensor(
                        out=mb_u[:, c0:c0 + 512], in0=mb_u[:, c0:c0 + 512],
                        scalar=0.125, in1=p2[:], op0=MUL, op1=SUB)
                    nc.gpsimd.dma_start(out=x1b_d[:, c0:c0 + 512],
                                        in_=x1b_u[7:NQ, c0:c0 + 512])
                    nc.gpsimd.dma_start(out=MBD[:, c0:c0 + 512],
                                        in_=mb_u[7:NQ, c0:c0 + 512])

                # m_a from the early-emitted doubling results
                nc.scalar.square(ma[:, 1:1033], x1a[:, 1:1033])
                nc.vector.scalar_tensor_tensor(
                    out=ma[:, 1:1033], in0=ma[:, 1:1033], scalar=0.125,
                    in1=Y2A[:, 1:1033], op0=MUL, op1=SUB)

                return {
                    "views": [
                        (mc_u[:], x1c_u[:]),
                        (mb_u[0:NO, :], x1b_u[0:NO, :]),
                        (me_u[:], x1e_u[:]),
                        (ma[:, 1:1025], x1a[:, 1:1025]),
                        (ma[:, 8:1032], x1a[:, 8:1032]),
                        (me_d[:], x1e_d[:]),
                        (MBD[:], x1b_d[:]),
                        (mc_d[:], x1c_d[:]),
                    ]}

            def emit_doubling(s, t):
                xs2 = t["xs2"]
                x1a = pres.tile([NO, WS], F32, name=f"x1a{s}", tag="x1a",
                                bufs=2)
                ma = pres.tile([NO, WS], F32, name=f"ma{s}", tag="ma", bufs=2)
                for i_d, (src, dst) in enumerate(((xs2, x1a), (xs2, Y2A))):
                    if i_d == 1:
                        nc.scalar.square(xs2[:], xs2[:])
                    nc.vector.tensor_add(SC1[:, 1:1039], src[:, 1:1039],
                                         src[:, 2:1040])
                    nc.vector.tensor_add(SC2[:, 1:1037], SC1[:, 1:1037],
                                         SC1[:, 3:1039])
                    nc.vector.tensor_add(dst[:, 1:1033], SC2[:, 1:1033],
                                         SC2[:, 5:1037])
                return x1a, ma

            def chain(s, r):
                views = r["views"]
                # max tree: e-dependent results (d2, d5) joined last
                nc.vector.tensor_tensor(T0[:], views[3][0], views[4][0], MAX)
                nc.vector.tensor_tensor(T1[:], views[1][0], views[6][0], MAX)
                nc.vector.tensor_tensor(T0[:], T0[:], T1[:], MAX)
                nc.vector.tensor_tensor(T1[:], views[0][0], views[7][0], MAX)
                nc.vector.tensor_tensor(T0[:], T0[:], T1[:], MAX)
                nc.vector.tensor_tensor(T1[:], views[2][0], views[5][0], MAX)
                nc.vector.tensor_tensor(T0[:], T0[:], T1[:], MAX)

                bx = pout.tile([NO, W], F32, name=f"bx{s}", tag="bx")
                outt = pout.tile([NO, W], F32, name=f"out{s}", tag="outt")
                nc.scalar.copy(bx[:], views[7][1])
                for d in range(6, -1, -1):
                    mk = (MK0 if d % 2 == 0 else MK1)[:]
                    nc.vector.tensor_tensor(mk, views[d][0], T0[:], EQ)
                    nc.vector.copy_predicated(bx[:], mk, views[d][1])
                nc.scalar.mul(outt[:], bx[:], 0.125)
                nout = SOUT if s < NS - 1 else W - (NS - 1) * SOUT
                nc.sync.dma_start(
                    out=bass.AP(tensor=yp.tensor,
                                offset=yp.offset + s * SOUT * W,
                                ap=[[W, nout], [1, W]]),
                    in_=outt[0:nout, :])

            t = loads(0)
            dbl = emit_doubling(0, t)
            res = None
            for s in range(NS):
                t_next = loads(s + 1) if s + 1 < NS else None
                res = stripe(s, t, dbl)
                if t_next is not None:
                    dbl = emit_doubling(s + 1, t_next)
                chain(s, res)
                t = t_next
    nc.compile()
    return nc


def _make_band():
    wb = np.zeros((128, NQ), np.float32)
    for q in range(NQ):
        wb[q:q + 8, q] = 1.0
    return wb


_nc_cache = []


def _get_nc():
    if not _nc_cache:
        _nc_cache.append(_build())
    return _nc_cache[0]


def _in_maps(x):
    wb = _make_band()
    maps = []
    for i in range(x.shape[0]):
        xpad = np.zeros((HPAD, WPAD), np.float32)
        xpad[7:7 + W, PADL:PADL + W] = x[i, 0]
        maps.append({"x": xpad, "wb": wb})
    return maps


def kernel(x, weight=None, _want_results=False, **_ignored):
    x = np.ascontiguousarray(np.asarray(x), dtype=np.float32)
    n = x.shape[0]
    assert x.shape == (n, 1, W, W), x.shape
    nc = _get_nc()
    res = run_bass_kernel_spmd(nc, _in_maps(x), core_ids=list(range(n)))
    out = np.stack([r["y"] for r in res.results])[:, None]
    if _want_results:
        return out, res
    return out


if __name__ == "__main__":
    rng = np.random.default_rng(0)
    x = rng.standard_normal((8, 1, W, W)).astype(np.float32)
    y = kernel(x)
    print("ran; out shape", y.shape, "mean", y.mean())


# revision 7
# speedup vs baseline: 1.1491x; 1.1400x over previous
"""Trainium2 Bass kernel for the directional min-variance filter (Kuwahara-style).

Row-per-partition, PE-centric design. The 1024x1024 image is processed in 9
stripes of 114 output rows. Per stripe, a 128-row x tile is DMA-loaded three
ways from a host-padded DRAM image: straight, sheared +1 col/row, sheared -1
col/row. A banded 128x121 fp32 matmul (8-ones diagonal band) computes 8-pixel
window sums along the partition axis, which in the three layouts realizes the
vertical, diagonal, and anti-diagonal ray sums of x and x^2 on the otherwise
idle Tensor engine. Horizontal ray sums run on the Vector engine via log2
doubling on a separate 114-row tile. Sheared results are un-sheared by a DMA
roundtrip through DRAM (parallelogram write, straight read), which also
provides the two partition origins (ray rows r0-7.. and r0..) that the
selection step needs, since compute engines require partition-0-aligned
accesses.

Metric per forward direction i: m_i = x1_i^2/8 - y2_i (maximize == minimize
variance); backward rays reuse the forward buffers via shifted views.
Selection: best metric M via a 7-op max tree, then first-wins payload select
with is_equal masks + copy_predicated in reverse direction order (exact
argmin tie semantics). 8 NeuronCores run data-parallel over the 8 images.
"""

import numpy as np

import concourse.bass as bass
import concourse.bacc as bacc
import concourse.tile as tile
from concourse import mybir
from concourse.bass_utils import run_bass_kernel_spmd

F32 = mybir.dt.float32
U8 = mybir.dt.uint8

W = 1024
SOUT = 114                  # output rows per stripe (last stripe: 112)
NS = 9
NQ = 121                    # metric rows per stripe (ray rows r0-7 .. r0+113)
NO = 114                    # chain rows per stripe
PADL = 144
PADR = 152
WPAD = PADL + W + PADR      # 1320
HPAD = 1040                 # xpad rows: image rows -7 .. 1032
WS = W + 16                 # straight tiles: col j <-> image col j-8
WZ = W + 136                # sheared tiles
WSCR = 1312                 # DRAM unshear scratch row pitch (>= NQ + WZ + 24)

MAX = mybir.AluOpType.max
EQ = mybir.AluOpType.is_equal
MUL = mybir.AluOpType.mult
SUB = mybir.AluOpType.subtract


def _build():
    nc = bacc.Bacc("TRN2", target_bir_lowering=False)
    x_t = nc.declare_dram_parameter("x", [HPAD, WPAD], F32, isOutput=False)
    wb_t = nc.declare_dram_parameter("wb", [128, NQ], F32, isOutput=False)
    y_t = nc.declare_dram_parameter("y", [W, W], F32, isOutput=True)
    scr_t = nc.dram_tensor("scr", [4, NQ, WSCR], F32, kind="Internal")
    xp = x_t[:]
    yp = y_t[:]
    scr = scr_t[:]

    def sbase(bi):
        return scr.offset + bi * NQ * WSCR

    with tile.TileContext(nc) as tc:
        with tc.tile_pool(name="const", bufs=1) as pc, \
             tc.tile_pool(name="pin", bufs=2) as pin, \
             tc.tile_pool(name="pres", bufs=1) as pres, \
             tc.tile_pool(name="pscr", bufs=1) as pscr, \
             tc.tile_pool(name="pout", bufs=2) as pout, \
             tc.tile_pool(name="pps", bufs=1, space="PSUM") as pps:

            band = pc.tile([128, NQ], F32)
            nc.sync.dma_start(out=band[:], in_=wb_t[:])

            SC1 = pscr.tile([NO, WS], F32)
            SC2 = pscr.tile([NO, WS], F32)
            Y2A = pscr.tile([NO, WS], F32)
            T0 = pscr.tile([NO, W], F32)
            T1 = pscr.tile([NO, W], F32)
            MBD = pscr.tile([NO, W], F32)
            MK0 = pscr.tile([NO, W], U8)
            MK1 = pscr.tile([NO, W], U8)

            def loads(s):
                r0 = s * SOUT
                t = {}
                t["xs"] = pin.tile([128, WS], F32, name=f"xs{s}", tag="xs")
                t["xs2"] = pin.tile([NO, WS], F32, name=f"xs2{s}", tag="xs2")
                t["zp"] = pin.tile([128, WZ], F32, name=f"zp{s}", tag="zp")
                t["zm"] = pin.tile([128, WZ], F32, name=f"zm{s}", tag="zm")
                nc.sync.dma_start(
                    out=t["xs"][:], in_=bass.AP(
                        tensor=xp.tensor, offset=xp.offset + r0 * WPAD + PADL - 8,
                        ap=[[WPAD, 128], [1, WS]]))
                nc.sync.dma_start(
                    out=t["xs2"][:], in_=bass.AP(
                        tensor=xp.tensor,
                        offset=xp.offset + (r0 + 7) * WPAD + PADL - 8,
                        ap=[[WPAD, NO], [1, WS]]))
                nc.sync.dma_start(
                    out=t["zp"][:], in_=bass.AP(
                        tensor=xp.tensor, offset=xp.offset + r0 * WPAD + PADL - 128,
                        ap=[[WPAD + 1, 128], [1, WZ]]))
                nc.sync.dma_start(
                    out=t["zm"][:], in_=bass.AP(
                        tensor=xp.tensor, offset=xp.offset + r0 * WPAD + PADL - 8,
                        ap=[[WPAD - 1, 128], [1, WZ]]))
                return t

            def stripe(s, t, dbl_done):
                xs, xs2, zp, zm = t["xs"], t["xs2"], t["zp"], t["zm"]

                def rt(nm, p, w):
                    return pres.tile([p, w], F32, name=f"{nm}{s}", tag=nm)

                x1a, ma = dbl_done  # from emit_doubling
                x1b_u = rt("x1b_u", NQ, W)
                mb_u = rt("mb_u", NQ, W)
                x1b_d = rt("x1b_d", NO, W)
                x1c_u = rt("x1c_u", NO, W)
                mc_u = rt("mc_u", NO, W)
                x1c_d = rt("x1c_d", NO, W)
                mc_d = rt("mc_d", NO, W)
                x1e_u = rt("x1e_u", NO, W)
                me_u = rt("me_u", NO, W)
                x1e_d = rt("x1e_d", NO, W)
                me_d = rt("me_d", NO, W)
                x1sh = pres.tile([NQ, WZ], F32, name=f"x1sh{s}", tag="x1sh", bufs=2)
                msh = pres.tile([NQ, WZ], F32, name=f"msh{s}", tag="msh", bufs=2)
                x1sh2 = pres.tile([NQ, WZ], F32, name=f"x1sh2{s}", tag="x1sh2", bufs=2)
                msh2 = pres.tile([NQ, WZ], F32, name=f"msh2{s}", tag="msh2", bufs=2)

                # --- sheared dirs first (c: +1, e: -1); chunked roundtrip ---
                for k, zt, x1t, mt, sb, sadj, woff, rds in (
                        ("c", zp, x1sh, msh, 0, 1, 0,
                         ((1, 0, 121), (1, 7, 128), (0, 0, 121), (0, 7, 128))),
                        ("e", zm, x1sh2, msh2, 2, -1, 120,
                         ((3, 0, 135), (3, 7, 128), (2, 0, 135), (2, 7, 128)))):
                    for c0 in (0, 512, 1024):
                        cw = min(512, WZ - c0)
                        p1 = pps.tile([NQ, 512], F32, name=f"px{k}{s}{c0}",
                                      tag="psC", bufs=2)
                        nc.tensor.matmul(p1[:, :cw], band[:], zt[:, c0:c0 + cw],
                                         start=True, stop=True)
                        nc.scalar.copy(x1t[:, c0:c0 + cw], p1[:, :cw])
                        nc.scalar.square(mt[:, c0:c0 + cw], p1[:, :cw])
                        # x1 sheared chunk can roundtrip-write immediately
                        nc.gpsimd.dma_start(
                            out=bass.AP(tensor=scr.tensor,
                                        offset=sbase(sb) + woff + c0,
                                        ap=[[WSCR + sadj, NQ], [1, cw]]),
                            in_=x1t[:, c0:c0 + cw])
                    nc.scalar.square(zt[:], zt[:])  # in-place x^2
                    for c0 in (0, 512, 1024):
                        cw = min(512, WZ - c0)
                        p2 = pps.tile([NQ, 512], F32, name=f"py{k}{s}{c0}",
                                      tag="psD", bufs=2)
                        nc.tensor.matmul(p2[:, :cw], band[:], zt[:, c0:c0 + cw],
                                         start=True, stop=True)
                        nc.vector.scalar_tensor_tensor(
                            out=mt[:, c0:c0 + cw], in0=mt[:, c0:c0 + cw],
                            scalar=0.125, in1=p2[:, :cw], op0=MUL, op1=SUB)
                        nc.gpsimd.dma_start(
                            out=bass.AP(tensor=scr.tensor,
                                        offset=sbase(sb + 1) + woff + c0,
                                        ap=[[WSCR + sadj, NQ], [1, cw]]),
                            in_=mt[:, c0:c0 + cw])
                    # row-split straight reads (2-way DMA parallelism)
                    dsts = {("c"): (mc_u, mc_d, x1c_u, x1c_d),
                            ("e"): (me_u, me_d, x1e_u, x1e_d)}[k]
                    for dst, (bi_l, roff, coff) in zip(dsts, rds):
                        for rr0, rr1 in ((0, 57), (57, NO)):
                            nc.gpsimd.dma_start(
                                out=dst[rr0:rr1, :], in_=bass.AP(
                                    tensor=scr.tensor,
                                    offset=(sbase(bi_l) + (roff + rr0) * WSCR
                                            + coff),
                                    ap=[[WSCR, rr1 - rr0], [1, W]]))

                # --- vertical (dir b) on PE; xs squared in-place after x1 ---
                pxv = []
                for c0 in (0, 512):
                    p1 = pps.tile([NQ, 512], F32, name=f"pxv{s}{c0}",
                                  tag="psA", bufs=2)
                    nc.tensor.matmul(p1[:], band[:], xs[:, 8 + c0:8 + c0 + 512],
                                     start=True, stop=True)
                    nc.scalar.copy(x1b_u[:, c0:c0 + 512], p1[:])
                    nc.scalar.square(mb_u[:, c0:c0 + 512], p1[:])
                nc.scalar.square(xs[:], xs[:])
                for c0 in (0, 512):
                    p2 = pps.tile([NQ, 512], F32, name=f"pyv{s}{c0}",
                                  tag="psB", bufs=2)
                    nc.tensor.matmul(p2[:], band[:], xs[:, 8 + c0:8 + c0 + 512],
                                     start=True, stop=True)
                    nc.vector.scalar_tensor_tensor(
                        out=mb_u[:, c0:c0 + 512], in0=mb_u[:, c0:c0 + 512],
                        scalar=0.125, in1=p2[:], op0=MUL, op1=SUB)
                    nc.gpsimd.dma_start(out=x1b_d[:, c0:c0 + 512],
                                        in_=x1b_u[7:NQ, c0:c0 + 512])
                    nc.gpsimd.dma_start(out=MBD[:, c0:c0 + 512],
                                        in_=mb_u[7:NQ, c0:c0 + 512])

                # m_a from the early-emitted doubling results
                nc.scalar.square(ma[:, 1:1033], x1a[:, 1:1033])
                nc.vector.scalar_tensor_tensor(
                    out=ma[:, 1:1033], in0=ma[:, 1:1033], scalar=0.125,
                    in1=Y2A[:, 1:1033], op0=MUL, op1=SUB)

                return {
                    "views": [
                        (mc_u[:], x1c_u[:]),
                        (mb_u[0:NO, :], x1b_u[0:NO, :]),
                        (me_u[:], x1e_u[:]),
                        (ma[:, 1:1025], x1a[:, 1:1025]),
                        (ma[:, 8:1032], x1a[:, 8:1032]),
                        (me_d[:], x1e_d[:]),
                        (MBD[:], x1b_d[:]),
                        (mc_d[:], x1c_d[:]),
                    ]}

            def emit_doubling(s, t):
                xs2 = t["xs2"]
                x1a = pres.tile([NO, WS], F32, name=f"x1a{s}", tag="x1a",
                                bufs=2)
                ma = pres.tile([NO, WS], F32, name=f"ma{s}", tag="ma", bufs=2)
                for i_d, (src, dst) in enumerate(((xs2, x1a), (xs2, Y2A))):
                    if i_d == 1:
                        nc.scalar.square(xs2[:], xs2[:])
                    nc.vector.tensor_add(SC1[:, 1:1039], src[:, 1:1039],
                                         src[:, 2:1040])
                    nc.vector.tensor_add(SC2[:, 1:1037], SC1[:, 1:1037],
                                         SC1[:, 3:1039])
                    nc.vector.tensor_add(dst[:, 1:1033], SC2[:, 1:1033],
                                         SC2[:, 5:1037])
                return x1a, ma

            def chain(s, r):
                views = r["views"]
                # max tree: e-dependent results (d2, d5) joined last
                nc.vector.tensor_tensor(T0[:], views[3][0], views[4][0], MAX)
                nc.vector.tensor_tensor(T1[:], views[1][0], views[6][0], MAX)
                nc.vector.tensor_tensor(T0[:], T0[:], T1[:], MAX)
                nc.vector.tensor_tensor(T1[:], views[0][0], views[7][0], MAX)
                nc.vector.tensor_tensor(T0[:], T0[:], T1[:], MAX)
                nc.vector.tensor_tensor(T1[:], views[2][0], views[5][0], MAX)
                nc.vector.tensor_tensor(T0[:], T0[:], T1[:], MAX)

                bx = pout.tile([NO, W], F32, name=f"bx{s}", tag="bx")
                outt = pout.tile([NO, W], F32, name=f"out{s}", tag="outt")
                nc.scalar.copy(bx[:], views[7][1])
                for d in range(6, -1, -1):
                    mk = (MK0 if d % 2 == 0 else MK1)[:]
                    nc.vector.tensor_tensor(mk, views[d][0], T0[:], EQ)
                    nc.vector.copy_predicated(bx[:], mk, views[d][1])
                nc.scalar.mul(outt[:], bx[:], 0.125)
                nout = SOUT if s < NS - 1 else W - (NS - 1) * SOUT
                nc.sync.dma_start(
                    out=bass.AP(tensor=yp.tensor,
                                offset=yp.offset + s * SOUT * W,
                                ap=[[W, nout], [1, W]]),
                    in_=outt[0:nout, :])

            t = loads(0)
            dbl = emit_doubling(0, t)
            res = None
            for s in range(NS):
                t_next = loads(s + 1) if s + 1 < NS else None
                res = stripe(s, t, dbl)
                if t_next is not None:
                    dbl = emit_doubling(s + 1, t_next)
                chain(s, res)
                t = t_next
    nc.compile()
    return nc


def _make_band():
    wb = np.zeros((128, NQ), np.float32)
    for q in range(NQ):
        wb[q:q + 8, q] = 1.0
    return wb


_nc_cache = []


def _get_nc():
    if not _nc_cache:
        _nc_cache.append(_build())
    return _nc_cache[0]


def _in_maps(x):
    wb = _make_band()
    maps = []
    for i in range(x.shape[0]):
        xpad = np.zeros((HPAD, WPAD), np.float32)
        xpad[7:7 + W, PADL:PADL + W] = x[i, 0]
        maps.append({"x": xpad, "wb": wb})
    return maps


def kernel(x, weight=None, _want_results=False, **_ignored):
    x = np.ascontiguousarray(np.asarray(x), dtype=np.float32)
    n = x.shape[0]
    assert x.shape == (n, 1, W, W), x.shape
    nc = _get_nc()
    res = run_bass_kernel_spmd(nc, _in_maps(x), core_ids=list(range(n)))
    out = np.stack([r["y"] for r in res.results])[:, None]
    if _want_results:
        return out, res
    return out


if __name__ == "__main__":
    rng = np.random.default_rng(0)
    x = rng.standard_normal((8, 1, W, W)).astype(np.float32)
    y = kernel(x)
    print("ran; out shape", y.shape, "mean", y.mean())
